# revision 3
# baseline (speedup 1.0000x reference)
"""Trainium2 Bass kernel for a basic tanh RNN + output projection.

Reference computation (all fp32):
    s_t = tanh(x[:, :, t] @ Wx + s_{t-1} @ Wh + b)      t = 0..T-1, s_{-1} = 0
    out[:, t, :] = s_t @ Wout + bout

Shapes: x (64, 256, 1024), Wx (256, 1024), Wh (1024, 1024), b (1024,),
        Wout (1024, 512), bout (512,)  ->  out (64, 1024, 512)

Strategy (8 NeuronCores):
  The T=1024 recurrence is sequential and, per step, the PE cost is dominated
  by stationary-weight loads (64 Wh tiles of 128x128 every step), which is the
  same whether a core carries 8 or 64 batch rows.  So every core runs the
  full-batch recurrence (replicated; state kept transposed [H, B] so no
  per-step transposes are needed), and the parallel work -- the output
  projection and the output writes -- is sharded by batch.  Each core receives
  x with the batch axis rotated so that its own 8 batch columns sit at
  positions 0..7; all cores then run one identical program.
"""

import numpy as np
import ml_dtypes

import concourse.bass as bass
from concourse import bacc
import concourse.mybir as mybir
import concourse.tile as tile
from concourse.bass_utils import run_bass_kernel_spmd

B, F, T = 64, 256, 1024
H, O = 1024, 512
NCORES = 8
MB = B // NCORES  # own-batch columns per core (projection shard)
P = 128
KH, KF, MH, OBK = H // P, F // P, H // P, O // P  # 8, 2, 8, 4

BF16 = mybir.dt.bfloat16
F32 = mybir.dt.float32
np_bf16 = ml_dtypes.bfloat16


def build_program(t_steps: int = T, w_steps: int = 32) -> bass.Bass:
    assert t_steps % w_steps == 0
    nw = t_steps // w_steps
    pw = w_steps * MB  # projection moving size per window

    nc = bacc.Bacc()

    xt_d = nc.declare_dram_parameter("xt", [t_steps, F, B], BF16, isOutput=False)
    wh_d = nc.declare_dram_parameter("wh", [H, H], BF16, isOutput=False)
    wx_d = nc.declare_dram_parameter("wx", [F, H], BF16, isOutput=False)
    wo_d = nc.declare_dram_parameter("wout", [H, O], BF16, isOutput=False)
    b_d = nc.declare_dram_parameter("bvec", [H], F32, isOutput=False)
    bo_d = nc.declare_dram_parameter("boutvec", [O], F32, isOutput=False)
    out_d = nc.declare_dram_parameter("out", [nw, OBK, P, pw], F32, isOutput=True)

    with tile.TileContext(nc) as tc:
        with (
            tc.tile_pool(name="const", bufs=1) as cpool,
            tc.tile_pool(name="stage", bufs=2) as spool,
            tc.tile_pool(name="xin", bufs=6) as xpool,
            tc.tile_pool(name="outsb", bufs=4) as opool,
            tc.tile_pool(name="psz", bufs=4, space="PSUM") as zpool,
            tc.tile_pool(name="psp", bufs=2, space="PSUM") as ppool,
        ):
            # --- resident weights ---------------------------------------
            wh_sb = cpool.tile([P, KH, H], BF16, tag="wh")
            nc.sync.dma_start(wh_sb[:], wh_d.rearrange("(kb p) c -> p kb c", p=P))
            wx_sb = cpool.tile([P, KF, H], BF16, tag="wx")
            nc.sync.dma_start(wx_sb[:], wx_d.rearrange("(kb p) c -> p kb c", p=P))
            wo_sb = cpool.tile([P, MH, O], BF16, tag="wo")
            nc.sync.dma_start(wo_sb[:], wo_d.rearrange("(kb p) c -> p kb c", p=P))
            b_sb = cpool.tile([P, KH], F32, tag="b")
            nc.sync.dma_start(b_sb[:], b_d.rearrange("(m p) -> p m", p=P))
            bo_sb = cpool.tile([P, OBK], F32, tag="bo")
            nc.sync.dma_start(bo_sb[:], bo_d.rearrange("(m p) -> p m", p=P))

            stage_prev = None  # window t//w - 1 state tiles, [P, w, B] per m
            stage_cur = None
            pending_proj = []  # (window_idx, stage_tiles) awaiting projection

            def emit_proj_block(w_idx, stiles, ob):
                pp = ppool.tile([P, pw], F32, tag="pproj", name="pproj")
                for m in range(MH):
                    nc.tensor.matmul(
                        pp,
                        wo_sb[:, m, ob * P : (ob + 1) * P],
                        stiles[m][:, :, 0:MB],
                        start=(m == 0),
                        stop=(m == MH - 1),
                    )
                osb = opool.tile([P, pw], F32, tag="osb", name="osb")
                nc.scalar.activation(
                    osb,
                    pp,
                    mybir.ActivationFunctionType.Identity,
                    bias=bo_sb[:, ob : ob + 1],
                )
                nc.sync.dma_start(out_d[w_idx, ob], osb)

            for t in range(t_steps):
                tl = t % w_steps
                if tl == 0:
                    stage_prev = stage_cur
                    stage_cur = [
                        spool.tile([P, w_steps, B], BF16, tag=f"stage{m}", name=f"stage{m}")
                        for m in range(MH)
                    ]

                xt_sb = xpool.tile([P, KF, B], BF16, tag="xt", name="xt")
                nc.sync.dma_start(
                    xt_sb[:], xt_d[t].rearrange("(kb p) b -> p kb b", p=P)
                )

                for m in range(MH):
                    ps = zpool.tile([P, B], F32, tag="psz", name="psz")
                    nlast = KF - 1 if t == 0 else KF + KH - 1
                    idx = 0
                    for kf in range(KF):
                        nc.tensor.matmul(
                            ps,
                            wx_sb[:, kf, m * P : (m + 1) * P],
                            xt_sb[:, kf, :],
                            start=(idx == 0),
                            stop=(idx == nlast),
                        )
                        idx += 1
                    if t > 0:
                        prev = stage_cur if tl > 0 else stage_prev
                        ptl = (t - 1) % w_steps
                        for k in range(KH):
                            nc.tensor.matmul(
                                ps,
                                wh_sb[:, k, m * P : (m + 1) * P],
                                prev[k][:, ptl, :],
                                start=False,
                                stop=(idx == nlast),
                            )
                            idx += 1
                    nc.scalar.activation(
                        stage_cur[m][:, tl, :],
                        ps,
                        mybir.ActivationFunctionType.Tanh,
                        bias=b_sb[:, m : m + 1],
                    )

                # spread the previous window's projection over this window
                if pending_proj and tl % 2 == 1 and tl // 2 < OBK:
                    emit_proj_block(pending_proj[0][0], pending_proj[0][1], tl // 2)
                    if tl // 2 == OBK - 1:
                        pending_proj.pop(0)

                if tl == w_steps - 1:
                    pending_proj.append((t // w_steps, stage_cur))

            # drain remaining projections (the last window's)
            for w_idx, stiles in pending_proj:
                for ob in range(OBK):
                    emit_proj_block(w_idx, stiles, ob)

    nc.compile()
    return nc


def _host_prep(x, Wx, Wh, b, Wout, bout, t_steps):
    """Build the 8 per-core input maps."""
    xt = np.ascontiguousarray(x[:, :, :t_steps].transpose(2, 1, 0)).astype(np_bf16)
    wh = Wh.astype(np_bf16)
    wx = Wx.astype(np_bf16)
    wo = Wout.astype(np_bf16)
    bv = np.ascontiguousarray(b, dtype=np.float32)
    bo = np.ascontiguousarray(bout, dtype=np.float32)
    in_maps = []
    for c in range(NCORES):
        xt_c = np.ascontiguousarray(np.roll(xt, -MB * c, axis=2))
        in_maps.append(
            {
                "xt": xt_c,
                "wh": wh,
                "wx": wx,
                "wout": wo,
                "bvec": bv,
                "boutvec": bo,
            }
        )
    return in_maps


def _assemble(results, t_steps, w_steps):
    nw = t_steps // w_steps
    out = np.empty((B, t_steps, O), np.float32)
    for c in range(NCORES):
        arr = results[c]["out"].reshape(nw, OBK, P, w_steps, MB)
        # out[MB*c + j, w*W + tl, ob*P + p] = arr[w, ob, p, tl, j]
        out[MB * c : MB * (c + 1)] = (
            arr.transpose(4, 0, 3, 1, 2).reshape(MB, t_steps, O)
        )
    return out


def run(x, Wx, Wh, b, Wout, bout, t_steps=T, w_steps=32, trace=False):
    nc = build_program(t_steps, w_steps)
    in_maps = _host_prep(x, Wx, Wh, b, Wout, bout, t_steps)
    res = run_bass_kernel_spmd(nc, in_maps, list(range(NCORES)), trace=trace)
    out = _assemble(res.results, t_steps, w_steps)
    return out, res


def kernel(x, Wx, Wh, b, Wout, bout):
    out, _ = run(
        np.asarray(x, dtype=np.float32),
        np.asarray(Wx, dtype=np.float32),
        np.asarray(Wh, dtype=np.float32),
        np.asarray(b, dtype=np.float32),
        np.asarray(Wout, dtype=np.float32),
        np.asarray(bout, dtype=np.float32),
    )
    return out
